# revision 32
# baseline (speedup 1.0000x reference)
"""Trainium2 Bass kernel for nn_Actor_87497073754359.

Math (per batch b of B=128, x[b] is [N=2048, D=128] f32):
  graph_emb = mean_n x[b];  first/curr = x[b, idx]
  q = Wq @ (W_lin @ concat(graph_emb, first, curr) + b_lin) + bq  -> [H=8, HD=16]
  scores[h, n] = q[h] . (x @ Wk.T)[n, h*16:+16] / 4 ; mask; softmax over n
  out[b] = mean_h softmax

Never materialize k = x@Wk.T. Fold q into Wk:
  t[b][c, h] = sum_j Wk[j, c] * headsel_h(j) * q[b, j] * 0.25
  scores[b][h, n] = sum_c t[b][c, h] * xT[b][c, n]
x streams once as a host-pretransposed bf16 copy. All 16 row-sums for
the mean run on the TensorEngine (pointwise psum-accumulate with an
identity stationary + short DVE reduce) — this both uses the fastest
engine for the reduction and keeps the PE HAM clock warm (2.4 GHz);
a ~5us warm-up matmul burst at kernel start trips HAM before real work.
1/N is folded into the host-combined Wq@W_lin weight.

Sharding: pure data parallel over batch (16/core), no collectives.
"""

import numpy as np
import ml_dtypes

import concourse.bass as bass
import concourse.tile as tile
from concourse import bacc, mybir
from concourse.bass_utils import run_bass_kernel_spmd
from concourse.masks import make_identity

B, N, D, H = 128, 2048, 128, 8
HD = D // H
NCORES = 8
BPC = B // NCORES          # 16 batches per core
P = 128
CH = 512                   # psum-bank chunk of n
NCH = N // CH              # 4
NG = 4                     # batch groups per core
GS = BPC // NG             # 4 batches per group
FOLD = 16                  # accumulate-DMA fold factor for row sums
MASKVAL = -1000.0          # exp(-1000 + s) == 0.0 exactly in f32

BF16 = mybir.dt.bfloat16
F32 = mybir.dt.float32
I32 = mybir.dt.int32


def build_kernel_body(ctx, tc):
    nc = tc.nc

    # ---- DRAM parameters (per-core shapes) ----
    xt = nc.dram_tensor("xt", [BPC, P, N], BF16, kind="ExternalInput")
    xn = nc.dram_tensor("xn", [BPC * N, D], BF16, kind="ExternalInput")
    gidx = nc.dram_tensor("gidx", [2 * BPC, 1], I32, kind="ExternalInput")
    maskneg = nc.dram_tensor("maskneg", [BPC, N], BF16, kind="ExternalInput")
    indmask = nc.dram_tensor("indmask", [BPC + 1, NG * P], BF16, kind="ExternalInput")
    ind16 = nc.dram_tensor("ind16", [P, NG * BPC], BF16, kind="ExternalInput")
    wcombt = nc.dram_tensor("wcombt", [3, P, D], BF16, kind="ExternalInput")
    wk = nc.dram_tensor("wk", [D, D], BF16, kind="ExternalInput")
    headind4 = nc.dram_tensor("headind4", [D, GS * H], BF16, kind="ExternalInput")
    biasq = nc.dram_tensor("biasq", [D, 1], F32, kind="ExternalInput")
    out = nc.dram_tensor("out", [BPC, N], F32, kind="ExternalOutput")

    consts = ctx.enter_context(tc.tile_pool(name="consts", bufs=1))
    xt_pool = ctx.enter_context(tc.tile_pool(name="xt", bufs=BPC))
    scratch = ctx.enter_context(tc.tile_pool(name="scratch", bufs=2))
    w_pool = ctx.enter_context(tc.tile_pool(name="w", bufs=NG * NCH))
    small = ctx.enter_context(tc.tile_pool(name="small", bufs=3))
    out_pool = ctx.enter_context(tc.tile_pool(name="outp", bufs=1))
    psum_small = ctx.enter_context(tc.tile_pool(name="ps_small", bufs=2, space="PSUM"))
    psum_scores = ctx.enter_context(
        tc.tile_pool(name="ps_scores", bufs=4, space="PSUM")
    )
    psum_out = ctx.enter_context(tc.tile_pool(name="ps_out", bufs=2, space="PSUM"))

    # ---- PE warm-up: ~5us of dense matmuls so HAM reaches 8/8 during DMA ----
    warm_src = consts.tile([P, CH], BF16)
    nc.vector.memset(warm_src, 1.0)

    def emit_warmups(lo, hi):
        for i in range(lo, hi):
            pw = psum_small.tile([P, CH], F32, tag="ps", name=f"warm{i}")
            nc.tensor.matmul(
                out=pw[:],
                lhsT=warm_src[:, :P],
                rhs=warm_src[:],
                start=True,
                stop=True,
            )

    emit_warmups(0, 12)

    # ---- x stream first on the sync queue ----
    xt_tiles = []
    for b in range(BPC):
        xtb_t = xt_pool.tile([P, N], BF16, tag="xt", name=f"xt{b}")
        nc.sync.dma_start(xtb_t, xt[b])
        xt_tiles.append(xtb_t)

    # ---- constants into SBUF (scalar queue) ----
    wcombt_sb = consts.tile([P, 3, D], BF16)
    nc.scalar.dma_start(wcombt_sb, wcombt[:].rearrange("p c j -> c p j"))
    wk_sb = consts.tile([D, D], BF16)
    nc.scalar.dma_start(wk_sb, wk[:])
    headind4_sb = consts.tile([D, GS * H], BF16)
    nc.scalar.dma_start(headind4_sb, headind4[:])
    biasq_sb = consts.tile([D, 1], F32)
    nc.scalar.dma_start(biasq_sb, biasq[:])
    ind16_sb = consts.tile([P, NG * BPC], BF16)
    nc.scalar.dma_start(ind16_sb, ind16[:])

    indmask_sb = consts.tile([P, NG * P], BF16)
    nc.vector.memset(indmask_sb, 0.0)
    nc.scalar.dma_start(indmask_sb[: BPC + 1, :], indmask[:])

    maskneg_sb = consts.tile([P, N], BF16)
    nc.vector.memset(maskneg_sb, 0.0)
    nc.scalar.dma_start(maskneg_sb[:BPC, :], maskneg[:])

    gidx_sb = consts.tile([2 * BPC, 1], I32)
    nc.scalar.dma_start(gidx_sb, gidx[:])

    ident32 = consts.tile([2 * BPC, 2 * BPC], BF16)
    make_identity(nc, ident32[:])
    ident128 = consts.tile([P, P], BF16)
    make_identity(nc, ident128[:])

    # ---- gather first/current node rows: [32, 128] -> featsT [128, 32] bf16 ----
    grows = consts.tile([2 * BPC, D], BF16)
    nc.gpsimd.indirect_dma_start(
        out=grows[:],
        out_offset=None,
        in_=xn[:],
        in_offset=bass.IndirectOffsetOnAxis(ap=gidx_sb[:, :1], axis=0),
    )
    psum_f = psum_small.tile([P, 2 * BPC], BF16, space="PSUM", tag="ps")
    nc.tensor.transpose(psum_f[:], grows[:], ident32[:])
    featsT_sb = consts.tile([P, 2 * BPC], BF16)
    nc.vector.tensor_copy(featsT_sb[:], psum_f[:])

    # ---- row sums: PE b0-3 (psum-accumulate), ACT b4-7, DVE b8-9, DMA-fold b10-15
    sums_f32 = consts.tile([P, BPC], F32)
    sums_bf = consts.tile([P, BPC], BF16)

    def emit_mean_pe(b):
        psum_m = psum_scores.tile(
            [P, CH], F32, space="PSUM", tag="psum_s", name=f"pm{b}"
        )
        for j in range(8):
            nc.tensor.matmul(
                out=psum_m[:, :256],
                lhsT=ident128[:],
                rhs=xt_tiles[b][:, j * 256 : (j + 1) * 256],
                start=(j == 0),
                stop=(j == 7),
            )
        nc.vector.tensor_reduce(
            out=sums_f32[:, b : b + 1],
            in_=psum_m[:, :256],
            axis=mybir.AxisListType.X,
            op=mybir.AluOpType.add,
        )

    def emit_mean_act(b):
        scr = scratch.tile([P, N], BF16, tag="scr")
        nc.scalar.activation(
            out=scr[:],
            in_=xt_tiles[b][:],
            func=mybir.ActivationFunctionType.Copy,
            accum_out=sums_f32[:, b : b + 1],
        )

    def emit_mean_dve(b):
        nc.vector.tensor_reduce(
            out=sums_f32[:, b : b + 1],
            in_=xt_tiles[b][:],
            axis=mybir.AxisListType.X,
            op=mybir.AluOpType.add,
        )

    def emit_mean_fold(b):
        fold = scratch.tile([P, N // 2], BF16, tag="fold")
        nc.gpsimd.dma_start(out=fold[:], in_=xt_tiles[b][:, N // 2 :])
        nc.gpsimd.dma_start(
            out=fold[:], in_=xt_tiles[b][:, : N // 2], accum_op=mybir.AluOpType.add
        )
        nc.vector.tensor_reduce(
            out=sums_f32[:, b : b + 1],
            in_=fold[:],
            axis=mybir.AxisListType.X,
            op=mybir.AluOpType.add,
        )

    MEAN_ENGINE = {b: "pe" for b in range(BPC)}

    def emit_means_for_group(g):
        for b in range(g * GS, (g + 1) * GS):
            kind = MEAN_ENGINE[b]
            if kind == "pe":
                emit_mean_pe(b)
            elif kind == "act":
                emit_mean_act(b)
            elif kind == "dve":
                emit_mean_dve(b)
            else:
                emit_mean_fold(b)

    # ---- per group: cast, q chain, scores, exp; means for g+1 after scores ----
    sums_sb = consts.tile([P, NG * NCH], F32)
    rmat = consts.tile([P, NG * BPC], BF16)
    totals = consts.tile([P, NG], F32)
    recips = consts.tile([P, NG], F32)
    w_all = []
    emit_means_for_group(0)
    for g in range(NG):
        b0 = g * GS
        nc.vector.tensor_copy(sums_bf[:, b0 : b0 + GS], sums_f32[:, b0 : b0 + GS])
        psum_q = psum_small.tile([P, GS], F32, space="PSUM", tag="ps")
        ctx_chunks = [
            sums_bf[:, b0 : b0 + GS],
            featsT_sb[:, b0 : b0 + GS],
            featsT_sb[:, BPC + b0 : BPC + b0 + GS],
        ]
        for pch in range(3):
            nc.tensor.matmul(
                out=psum_q[:],
                lhsT=wcombt_sb[:, pch, :],
                rhs=ctx_chunks[pch],
                start=(pch == 0),
                stop=(pch == 2),
            )
        qgrep = small.tile([P, GS, H], BF16, tag="qgrep")
        nc.vector.tensor_scalar(
            out=qgrep[:],
            in0=psum_q[:, :, None].to_broadcast([P, GS, H]),
            scalar1=biasq_sb[:, 0:1],
            scalar2=None,
            op0=mybir.AluOpType.add,
        )
        qm_all = small.tile([P, GS * H], BF16, tag="qm")
        nc.vector.tensor_tensor(
            out=qm_all[:],
            in0=headind4_sb[:],
            in1=qgrep[:].rearrange("p s h -> p (s h)"),
            op=mybir.AluOpType.mult,
        )
        psum_t = psum_small.tile([P, GS * H], F32, space="PSUM", tag="ps")
        nc.tensor.matmul(
            out=psum_t[:], lhsT=wk_sb[:], rhs=qm_all[:], start=True, stop=True
        )
        tpad_all = small.tile([P, GS * H], BF16, tag="tpad")
        nc.vector.tensor_copy(tpad_all[:], psum_t[:])
        tpads = [tpad_all[:, s * H : (s + 1) * H] for s in range(GS)]

        w_group = []
        for ch in range(NCH):
            psum_s = psum_scores.tile([P, CH], F32, space="PSUM", tag="psum_s")
            nc.tensor.matmul(
                out=psum_s[:],
                lhsT=indmask_sb[:, g * P : (g + 1) * P],
                rhs=maskneg_sb[:, ch * CH : (ch + 1) * CH],
                start=True,
                stop=False,
                skip_group_check=True,
            )
            for s in range(GS):
                nc.tensor.matmul(
                    out=psum_s[32 * s : 32 * s + H, :],
                    lhsT=tpads[s],
                    rhs=xt_tiles[b0 + s][:, ch * CH : (ch + 1) * CH],
                    start=False,
                    stop=(s == GS - 1),
                    skip_group_check=True,
                    tile_position=(0, 32 * s),
                )
            wt = w_pool.tile([P, CH], BF16, tag="w")
            nc.scalar.activation(
                out=wt[:],
                in_=psum_s[:],
                func=mybir.ActivationFunctionType.Exp,
                accum_out=sums_sb[:, NCH * g + ch : NCH * g + ch + 1],
            )
            w_group.append(wt)
        w_all.append(w_group)
        if g + 1 < NG:
            emit_means_for_group(g + 1)

    for g in range(NG):
        nc.vector.tensor_reduce(
            out=totals[:, g : g + 1],
            in_=sums_sb[:, NCH * g : NCH * (g + 1)],
            axis=mybir.AxisListType.X,
            op=mybir.AluOpType.add,
        )
        nc.vector.reciprocal(recips[:, g : g + 1], totals[:, g : g + 1])
        nc.vector.tensor_scalar(
            out=rmat[:, g * BPC : (g + 1) * BPC],
            in0=ind16_sb[:, g * BPC : (g + 1) * BPC],
            scalar1=recips[:, g : g + 1],
            scalar2=None,
            op0=mybir.AluOpType.mult,
        )
    out_sb = out_pool.tile([BPC, N], F32)
    for ch in range(NCH):
        psum_o = psum_out.tile([BPC, CH], F32, space="PSUM", tag="po")
        for g in range(NG):
            nc.tensor.matmul(
                out=psum_o[:],
                lhsT=rmat[:, g * BPC : (g + 1) * BPC],
                rhs=w_all[g][ch][:],
                start=(g == 0),
                stop=(g == NG - 1),
            )
        nc.scalar.copy(out_sb[:, ch * CH : (ch + 1) * CH], psum_o[:])
        nc.sync.dma_start(
            out[:, ch * CH : (ch + 1) * CH], out_sb[:, ch * CH : (ch + 1) * CH]
        )


_NC_CACHE = None


def build_nc():
    global _NC_CACHE
    if _NC_CACHE is not None:
        return _NC_CACHE
    from contextlib import ExitStack

    nc = bacc.Bacc("TRN2", target_bir_lowering=False, debug=False)
    with tile.TileContext(nc) as tc:
        with ExitStack() as ctx:
            build_kernel_body(ctx, tc)
    nc.compile()
    _NC_CACHE = nc
    return nc


def make_in_maps(x, first_node, current_node, mask, W_lin, b_lin, Wq, bq, Wk, bk):
    """Host-side sharding/layout prep. Returns list of 8 per-core input dicts."""
    x = np.asarray(x, dtype=np.float32)
    mask = np.asarray(mask)
    first_node = np.asarray(first_node).astype(np.int32)
    current_node = np.asarray(current_node).astype(np.int32)
    W_lin = np.asarray(W_lin, dtype=np.float32)
    b_lin = np.asarray(b_lin, dtype=np.float32)
    Wq = np.asarray(Wq, dtype=np.float32)
    bq_v = np.asarray(bq, dtype=np.float32)
    Wk = np.asarray(Wk, dtype=np.float32)

    xbf = x.astype(ml_dtypes.bfloat16)

    # replicated weights; 1/N for the mean is folded into Wcomb chunk 0
    wcomb = (Wq @ W_lin).astype(np.float32)            # [D, 3D]
    wcomb[:, :D] *= 1.0 / N
    wcombt = np.ascontiguousarray(wcomb.T.reshape(3, P, D)).astype(ml_dtypes.bfloat16)
    biasq = (Wq @ b_lin + bq_v).astype(np.float32).reshape(D, 1)
    wk_in = np.ascontiguousarray(Wk).astype(ml_dtypes.bfloat16)
    headind = np.zeros((D, H), dtype=np.float32)
    for j in range(D):
        headind[j, j // HD] = 1.0 / np.sqrt(HD)
    headind4 = np.tile(headind, (1, GS)).astype(ml_dtypes.bfloat16)

    # indmask[r, g*128 + p]: p = 32*s + h
    indmask = np.zeros((BPC + 1, NG * P), dtype=np.float32)
    for g in range(NG):
        for s in range(GS):
            for h in range(32):
                pcol = g * P + 32 * s + h
                if h < H:
                    indmask[g * GS + s, pcol] = 1.0
                else:
                    indmask[BPC, pcol] = 1.0
    indmask = indmask.astype(ml_dtypes.bfloat16)

    # ind16[p, g*16 + b']: weight 1/8 from psum row p of group g to batch b'
    ind16 = np.zeros((P, NG * BPC), dtype=np.float32)
    for g in range(NG):
        for s in range(GS):
            for h in range(H):
                ind16[32 * s + h, g * BPC + g * GS + s] = 1.0 / H
    ind16 = ind16.astype(ml_dtypes.bfloat16)

    in_maps = []
    for c in range(NCORES):
        lo = c * BPC
        xs = xbf[lo : lo + BPC]                               # [16, 2048, 128]
        xtc = np.ascontiguousarray(xs.transpose(0, 2, 1))     # [16, 128, 2048]
        xnc = np.ascontiguousarray(xs.reshape(BPC * N, D))
        gi = np.concatenate(
            [
                np.arange(BPC, dtype=np.int32) * N + first_node[lo : lo + BPC, 0],
                np.arange(BPC, dtype=np.int32) * N + current_node[lo : lo + BPC, 0],
            ]
        ).reshape(2 * BPC, 1).astype(np.int32)
        mneg = (mask[lo : lo + BPC].astype(np.float32) * MASKVAL).astype(
            ml_dtypes.bfloat16
        )
        in_maps.append(
            {
                "xt": xtc,
                "xn": xnc,
                "gidx": gi,
                "maskneg": mneg,
                "indmask": indmask,
                "ind16": ind16,
                "wcombt": wcombt,
                "wk": wk_in,
                "headind4": headind4,
                "biasq": biasq,
            }
        )
    return in_maps


def kernel(**inputs) -> np.ndarray:
    nc = build_nc()
    in_maps = make_in_maps(**inputs)
    res = run_bass_kernel_spmd(nc, in_maps, core_ids=list(range(NCORES)))
    outs = [np.asarray(res.results[c]["out"]) for c in range(NCORES)]
    return np.concatenate(outs, axis=0)
